# revision 53
# baseline (speedup 1.0000x reference)
"""Trainium2 Bass kernel for nn_NodeNet (GNN message passing).

Strategy: data-parallel over graphs across 8 NeuronCores. Host transposes
inputs into [feature, row] layouts so every DMA is contiguous; all matmuls
run in bf16 with transposed activations:
  node stage: dp[128, rows] -> MLP -> sum over datapoints -> feature_enc[64, G]
  edge stage: rhs = [fe (broadcast per graph); edge_attr^T] -> MLP -> out^T

fused3 (fast path, requires structured edges + all-zero biases, which is what
the reference's setup_inputs produces):
  - one merged PSUM tile per MLP layer ([128, 2, TN] spanning 2 banks) so the
    psum->sbuf relu is a single instruction per layer instead of two
  - relu/copy work greedily load-balanced between ACT and DVE; the fe
    broadcast runs mostly on the (otherwise idle) gpsimd/Pool engine
  - all weights packed into one DRAM tensor -> single startup DMA
  - software pipeline emits [L3(t-1), L2(t), L1(t+2)] with DMA/broadcast prep
    running 3 steps ahead, so the PE never waits on elementwise producers
fused2 (previous structured path, used when biases are nonzero) and a fully
general two-launch fallback (arbitrary edge_index/batch) are kept below.
"""

import os
import sys

import ml_dtypes
import numpy as np

BF16NP = ml_dtypes.bfloat16

if "/opt/trn_rl_repo" not in sys.path and os.path.isdir("/opt/trn_rl_repo"):
    sys.path.insert(0, "/opt/trn_rl_repo")

import concourse.bacc as bacc
import concourse.tile as tile
from concourse import mybir
from concourse.bass_utils import run_bass_kernel_spmd

G, ODE, NDATA, H, EA, EPG = 4096, 64, 32, 256, 64, 128
E = G * EPG
NCORES = 8
GC = G // NCORES           # graphs per core
RC = GC * NDATA            # node-MLP rows per core
EC = GC * EPG              # edges per core
TN = 512                   # tile free size
CH = 64                    # graphs per interleave chunk
NCH = GC // CH             # chunks per core (8)
NNI = CH * NDATA // TN     # node iters per chunk (4)
NEI = CH * EPG // TN       # edge iters per chunk (16)
GPEI = TN // EPG           # graphs per edge iter (4)
GPNI = TN // NDATA         # graphs per node iter (16)

F32 = mybir.dt.float32
BF16 = mybir.dt.bfloat16
RELU = mybir.ActivationFunctionType.Relu
IDENT = mybir.ActivationFunctionType.Identity
COPY = mybir.ActivationFunctionType.Copy
ADD = mybir.AluOpType.add
MAX = mybir.AluOpType.max
AXX = mybir.AxisListType.X

_PROGRAMS = {}
last_results = None


def _install_trace_shim():
    """Optional: make trace=True work by injecting antenv.axon_hooks."""
    import types

    if "antenv.axon_hooks" in sys.modules:
        return
    try:
        mod = types.ModuleType("antenv.axon_hooks")
        mod._hook = None
        mod.set_axon_ntff_profile_hook = lambda h: setattr(mod, "_hook", h)
        mod.get_axon_ntff_profile_hook = lambda: mod._hook
        sys.modules["antenv.axon_hooks"] = mod
        import antenv

        antenv.axon_hooks = mod
        from trn_agent_boot.trn_boot import _ntff_profile_via_ctypes

        hook = _ntff_profile_via_ctypes("/opt/axon/libaxon_pjrt.so")
        if hook is not None:
            mod.set_axon_ntff_profile_hook(hook)
    except Exception:
        pass


# ------------------------- fused3: zero-bias fast path -------------------------

# packed weight column offsets: nw1 | nw2(k,m) | nw3(k) | ew1 | ew2(k,m) | ew3(k)
_W_NW1 = 0
_W_NW2 = 256
_W_NW3 = 768
_W_EW1 = 896
_W_EW2 = 1152
_W_EW3 = 1664
_W_COLS = 1792


def _build_fused3():
    nc = bacc.Bacc("TRN2", target_bir_lowering=False)
    wp_d = nc.dram_tensor("wpack", [128, _W_COLS], BF16, kind="ExternalInput")
    xT_d = nc.dram_tensor("xT", [128, RC], BF16, kind="ExternalInput")
    attrT_d = nc.dram_tensor("attrT", [64, EC], BF16, kind="ExternalInput")
    outT_d = nc.dram_tensor("outT", [64, EC], BF16, kind="ExternalOutput")

    with tile.TileContext(nc) as tc:
        with (
            tc.tile_pool(name="consts", bufs=1) as consts,
            tc.tile_pool(name="xin", bufs=8) as xin,
            tc.tile_pool(name="hid", bufs=3) as hid,
            tc.tile_pool(name="hsp", bufs=2) as hsp,
            tc.tile_pool(name="oot", bufs=4) as oot,
            tc.tile_pool(name="ps1", bufs=1, space="PSUM") as ps1,
            tc.tile_pool(name="ps2", bufs=2, space="PSUM") as ps2,
            tc.tile_pool(name="ps3", bufs=2, space="PSUM") as ps3,
        ):
            wp = consts.tile([128, _W_COLS], BF16, tag="wp", name="wp")
            nc.sync.dma_start(wp[:, 0:256], wp_d[:, 0:256])
            feT = consts.tile([ODE, GC], BF16, tag="feT")

            def w1(base, m):  # [128, 128] lhsT for layer-1 M-half m
                return wp[:, base + m * 128:base + (m + 1) * 128]

            def w2(base, k, m):
                return wp[:, base + k * 256 + m * 128:base + k * 256 + (m + 1) * 128]

            def w3(base, k):
                return wp[:, base + k * ODE:base + (k + 1) * ODE]

            # ---- interleaved iteration sequence ----
            seq = []

            def n_iter(c, i):
                return dict(kind="n", c=c, i=i, last=(i == NNI - 1))

            def e_iter(c, i):
                return dict(kind="e", c=c, i=i)

            for i in range(NNI):
                seq.append(n_iter(0, i))
            for c in range(NCH):
                nxt = list(range(NNI)) if c + 1 < NCH else []
                k = 0
                lead = 4 if c == 0 else 1
                for _ in range(lead):
                    if k < len(nxt):
                        seq.append(n_iter(c + 1, nxt[k]))
                        k += 1
                for i in range(NEI):
                    seq.append(e_iter(c, i))
                    if i % 4 == 3 and k < len(nxt):
                        seq.append(n_iter(c + 1, nxt[k]))
                        k += 1
                while k < len(nxt):
                    seq.append(n_iter(c + 1, nxt[k]))
                    k += 1

            # ---- ACT/DVE greedy load balancer ----
            bal = {"A": 0.0, "V": 0.0}

            def pick_engine(cost_a, cost_v):
                if bal["A"] + cost_a <= bal["V"] + cost_v:
                    bal["A"] += cost_a
                    return "A"
                bal["V"] += cost_v
                return "V"

            def emit_relu(out, in_, nels, force=None):
                ca = 220 + 0.85 * nels
                cv = 220 + 1.06 * nels
                if force == "A":
                    bal["A"] += ca
                    e = "A"
                elif force == "V":
                    bal["V"] += cv
                    e = "V"
                else:
                    e = pick_engine(ca, cv)
                if e == "A":
                    nc.scalar.activation(out, in_, RELU)
                else:
                    nc.vector.tensor_scalar(
                        out=out, in0=in_, scalar1=0.0, scalar2=None, op0=MAX)
                return "V" if e == "A" else "A"

            def emit_copy(out, in_, nels):
                bal["A"] += 220 + 0.85 * nels
                with nc.allow_low_precision(reason="bf16 out, zero bias"):
                    nc.scalar.activation(out, in_, COPY)

            rts = {}      # t -> input tile (node) / paired input tile (edge even i)
            h1s = {}      # t -> h1 sbuf tile
            h2s = {}      # t -> h2 sbuf tile
            l3ps = {}     # t (even local i) -> shared l3 psum tile
            hsums = {}
            nprep = [0]
            deferred = []

            def prep(t, step):
                d = seq[t]
                if d["kind"] == "n":
                    col0 = d["c"] * CH * NDATA + d["i"] * TN
                    xt = xin.tile([128, TN], BF16, tag="xt", name="xt")
                    nc.sync.dma_start(xt, xT_d[:, col0:col0 + TN])
                    rts[t] = xt
                elif d["i"] % 2 == 0:
                    e0 = d["c"] * CH * EPG + d["i"] * TN
                    g0 = d["c"] * CH + d["i"] * GPEI
                    rt = xin.tile([128, 2, TN], BF16, tag="rt", name="rt")
                    nc.sync.dma_start(
                        rt[64:128], attrT_d[:, e0:e0 + 2 * TN].rearrange(
                            "c (t e) -> c t e", t=2))
                    dst = rt[0:64].rearrange("c t (g e) -> c (t g) e", e=EPG)
                    src = feT[:, g0:g0 + 2 * GPEI, None].to_broadcast(
                        [ODE, 2 * GPEI, EPG])
                    nc.vector.tensor_copy(out=dst, in_=src)
                    nprep[0] += 1
                    rts[t] = rt

            def emit_l1(t, step):
                d = seq[t]
                n = d["kind"] == "n"
                base = _W_NW1 if n else _W_EW1
                if n:
                    mv = rts.pop(t)
                else:
                    j = d["i"] % 2
                    mv = rts[t - j][:, j]
                    if j == 1:
                        del rts[t - 1]
                l1 = ps1.tile([128, 2, TN], F32, tag="l1", name="l1")
                nc.tensor.matmul(l1[:, 0], w1(base, 0), mv, start=True, stop=True)
                nc.tensor.matmul(l1[:, 1], w1(base, 1), mv, start=True, stop=True)
                h1 = hid.tile([128, 2, TN], BF16, tag="h1", name="h1")
                d["h1eng"] = emit_relu(h1, l1, 1024, force="V")
                h1s[t] = h1

            def emit_l2(t, step):
                d = seq[t]
                n = d["kind"] == "n"
                base = _W_NW2 if n else _W_EW2
                h1 = h1s.pop(t)
                l2 = ps2.tile([128, 2, TN], F32, tag="l2", name="l2")
                for m in (0, 1):
                    for k in (0, 1):
                        nc.tensor.matmul(
                            l2[:, m], w2(base, k, m), h1[:, k],
                            start=(k == 0), stop=(k == 1))
                h2 = hid.tile([128, 2, TN], BF16, tag="h2", name="h2")
                emit_relu(h2, l2, 1024, force=d.get("h1eng"))
                if n:
                    c, i = d["c"], d["i"]
                    if i == 0:
                        hsums[c] = hsp.tile([128, 2, CH], BF16, tag="hsum",
                                            name="hsum")
                    h2r = h2.rearrange("c k (g d) -> c k g d", d=NDATA)
                    part = hid.tile([128, 2, GPNI, NDATA // 2], BF16,
                                    tag="part", name="part")
                    par2 = hid.tile([128, 2, GPNI, NDATA // 4], BF16,
                                    tag="par2", name="par2")
                    with nc.allow_low_precision(reason="bf16 partial sums"):
                        nc.gpsimd.tensor_tensor(
                            out=part, in0=h2r[:, :, :, 0:NDATA // 2],
                            in1=h2r[:, :, :, NDATA // 2:NDATA], op=ADD)
                        nc.gpsimd.tensor_tensor(
                            out=par2, in0=part[:, :, :, 0:NDATA // 4],
                            in1=part[:, :, :, NDATA // 4:NDATA // 2], op=ADD)
                        nc.vector.reduce_sum(
                            out=hsums[c][:, :, i * GPNI:(i + 1) * GPNI],
                            in_=par2.rearrange("c k g d -> c (k g) d"),
                            axis=AXX)
                h2s[t] = h2

            def emit_l3(t, step):
                d = seq[t]
                if d["kind"] == "n":
                    h2s.pop(t, None)
                    if d["last"]:
                        c = d["c"]
                        l3n = ps3.tile([128, TN], F32, tag="l3", name="l3")
                        for k in (0, 1):
                            nc.tensor.matmul(l3n[0:ODE, 0:CH], w3(_W_NW3, k),
                                             hsums[c][:, k],
                                             start=(k == 0), stop=(k == 1))
                        with nc.allow_low_precision(reason="bf16 feT"):
                            nc.scalar.activation(
                                feT[:, c * CH:(c + 1) * CH], l3n[0:ODE, 0:CH],
                                COPY)
                    return
                i = d["i"]
                j = i % 2
                if j == 0:
                    return
                h2a = h2s.pop(t - 1)
                h2b = h2s.pop(t)
                l3 = ps3.tile([128, TN], F32, tag="l3", name="l3")
                # same-weight matmuls adjacent: k0 over both pair halves, then k1
                for k in (0, 1):
                    nc.tensor.matmul(l3[0:64], w3(_W_EW3, k), h2a[:, k],
                                     start=(k == 0), stop=(k == 1))
                    nc.tensor.matmul(l3[64:128], w3(_W_EW3, k), h2b[:, k],
                                     start=(k == 0), stop=(k == 1))
                e0 = d["c"] * CH * EPG + i * TN
                ot = oot.tile([128, TN], BF16, tag="ot", name="ot")
                emit_copy(ot, l3, 512)
                nc.sync.dma_start(outT_d[:, e0 - TN:e0], ot[0:64])
                nc.sync.dma_start(outT_d[:, e0:e0 + TN], ot[64:128])

            NT = len(seq)
            for t in range(-4, NT + 1):
                if 0 <= t - 1 < NT:
                    emit_l3(t - 1, t)
                if 0 <= t + 4 < NT:
                    prep(t + 4, t)
                if 0 <= t < NT:
                    emit_l2(t, t)
                if 0 <= t + 2 < NT:
                    emit_l1(t + 2, t)
                if t == -2:
                    nc.sync.dma_start(wp[:, 256:_W_COLS], wp_d[:, 256:_W_COLS])
    nc.finalize()
    return nc


# ------------------------- fused2: structured path with biases ----------------

def _declare_weights(nc, with_eb3=True):
    t = {}
    t["nw1"] = nc.dram_tensor("nw1", [128, H], BF16, kind="ExternalInput")
    t["nw2"] = nc.dram_tensor("nw2", [128, 2, H], BF16, kind="ExternalInput")
    t["nw3"] = nc.dram_tensor("nw3", [128, 2, ODE], BF16, kind="ExternalInput")
    t["nb1"] = nc.dram_tensor("nb1", [128, 2], F32, kind="ExternalInput")
    t["nb2"] = nc.dram_tensor("nb2", [128, 2], F32, kind="ExternalInput")
    t["nb3"] = nc.dram_tensor("nb3", [ODE, 1], F32, kind="ExternalInput")
    t["ew1"] = nc.dram_tensor("ew1", [128, H], BF16, kind="ExternalInput")
    t["ew2"] = nc.dram_tensor("ew2", [128, 2, H], BF16, kind="ExternalInput")
    t["ew3"] = nc.dram_tensor("ew3", [128, 2, ODE], BF16, kind="ExternalInput")
    t["eb1"] = nc.dram_tensor("eb1", [128, 2], F32, kind="ExternalInput")
    t["eb2"] = nc.dram_tensor("eb2", [128, 2], F32, kind="ExternalInput")
    if with_eb3:
        t["eb3"] = nc.dram_tensor("eb3", [EA, 1], F32, kind="ExternalInput")
    return t


def _load_weights(nc, consts, td, node: bool, edge: bool, with_eb3=True):
    sb = {}
    names = []
    if node:
        names += ["nw1", "nw2", "nw3", "nb1", "nb2", "nb3"]
    if edge:
        names += ["ew1", "ew2", "ew3", "eb1", "eb2"]
        if with_eb3:
            names += ["eb3"]
    for n in names:
        d = td[n]
        sb[n] = consts.tile(list(d.shape), d.dtype, tag=n, name=n)
        nc.sync.dma_start(sb[n], d[:])
    return sb


def _build_fused2():
    """Structured path with bias support (see git history for details)."""
    nc = bacc.Bacc("TRN2", target_bir_lowering=False)
    td = _declare_weights(nc, with_eb3=False)
    xT_d = nc.dram_tensor("xT", [128, RC], BF16, kind="ExternalInput")
    attrT_d = nc.dram_tensor("attrT", [64, EC], BF16, kind="ExternalInput")
    outT_d = nc.dram_tensor("outT", [64, EC], BF16, kind="ExternalOutput")

    with tile.TileContext(nc) as tc:
        with (
            tc.tile_pool(name="consts", bufs=1) as consts,
            tc.tile_pool(name="xin", bufs=3) as xin,
            tc.tile_pool(name="hid", bufs=3) as hid,
            tc.tile_pool(name="oot", bufs=3) as oot,
            tc.tile_pool(name="hsp", bufs=2) as hsp,
            tc.tile_pool(name="ps1", bufs=2, space="PSUM") as ps1,
            tc.tile_pool(name="ps2", bufs=2, space="PSUM") as ps2,
            tc.tile_pool(name="ps3", bufs=2, space="PSUM") as ps3,
        ):
            w = _load_weights(nc, consts, td, node=True, edge=True, with_eb3=False)
            feT = consts.tile([ODE, GC], BF16, tag="feT")

            seq = []
            hsums = {}

            def add_node(c):
                for i in range(NNI):
                    seq.append(dict(kind="n", c=c, i=i,
                                    last=(i == NNI - 1)))

            add_node(0)
            for c in range(NCH):
                for q in range(4):
                    if c + 1 < NCH and not (c == 0 and q == 1):
                        seq.append(dict(kind="n", c=c + 1, i=q,
                                        last=(q == NNI - 1)))
                    if c == 0 and q == 0 and 1 < NCH:
                        seq.append(dict(kind="n", c=1, i=1, last=False))
                    for i in range(4 * q, 4 * q + 4):
                        seq.append(dict(kind="e", c=c, i=i))
                    if c == 0 and q == 0:
                        continue

            rts = {}
            h1s = {}
            l3ps = {}

            def prep(t):
                d = seq[t]
                if d["kind"] == "n":
                    col0 = d["c"] * CH * NDATA + d["i"] * TN
                    xt = xin.tile([128, TN], BF16, tag="xt", name="xt")
                    nc.sync.dma_start(xt, xT_d[:, col0:col0 + TN])
                    rts[t] = xt
                else:
                    e0 = d["c"] * CH * EPG + d["i"] * TN
                    g0 = d["c"] * CH + d["i"] * GPEI
                    rt = xin.tile([128, 2, TN], BF16, tag="rt", name="rt")
                    nc.sync.dma_start(
                        rt[64:128], attrT_d[:, e0:e0 + 2 * TN].rearrange(
                            "c (t e) -> c t e", t=2))
                    nc.vector.tensor_copy(
                        out=rt[0:64].rearrange("c t (g e) -> c (t g) e", e=EPG),
                        in_=feT[:, g0:g0 + 2 * GPEI, None].to_broadcast(
                            [ODE, 2 * GPEI, EPG]))
                    rts[t] = rt

            def emit_l1(t):
                d = seq[t]
                n = d["kind"] == "n"
                w1, b1 = (w["nw1"], w["nb1"]) if n else (w["ew1"], w["eb1"])
                if n:
                    prep(t)
                    mv = rts.pop(t)
                else:
                    if d["i"] % 2 == 0:
                        prep(t)
                    mv = rts[t - d["i"] % 2][:, d["i"] % 2]
                l1a = ps1.tile([128, TN], F32, tag="l1a", name="l1a", bufs=2)
                l1b = ps1.tile([128, TN], F32, tag="l1b", name="l1b", bufs=1)
                nc.tensor.matmul(l1a, w1[:, 0:128], mv, start=True, stop=True)
                nc.tensor.matmul(l1b, w1[:, 128:256], mv, start=True, stop=True)
                h1 = hid.tile([128, 2, TN], BF16, tag="h1", name="h1")
                nc.scalar.activation(h1[:, 0], l1a, RELU, bias=b1[:, 0:1])
                nc.vector.tensor_scalar(
                    out=h1[:, 1], in0=l1b, scalar1=b1[:, 1:2],
                    scalar2=0.0, op0=ADD, op1=MAX)
                h1s[t] = h1

            def emit_l2(t):
                d = seq[t]
                n = d["kind"] == "n"
                w2, b2 = (w["nw2"], w["nb2"]) if n else (w["ew2"], w["eb2"])
                h1 = h1s.pop(t)
                l2a = ps2.tile([128, TN], F32, tag="l2a", name="l2a", bufs=1)
                l2b = ps2.tile([128, TN], F32, tag="l2b", name="l2b", bufs=1)
                for m, lt in ((0, l2a), (1, l2b)):
                    for k in (0, 1):
                        nc.tensor.matmul(
                            lt, w2[:, k, m * 128:(m + 1) * 128],
                            h1[:, k], start=(k == 0), stop=(k == 1))
                h2 = hid.tile([128, 2, TN], BF16, tag="h2", name="h2")
                nc.scalar.activation(h2[:, 0], l2a, RELU, bias=b2[:, 0:1])
                if n and d["i"] % 2 == 0:
                    nc.scalar.activation(h2[:, 1], l2b, RELU,
                                         bias=b2[:, 1:2])
                else:
                    nc.vector.tensor_scalar(
                        out=h2[:, 1], in0=l2b, scalar1=b2[:, 1:2],
                        scalar2=0.0, op0=ADD, op1=MAX)
                if n:
                    c, i = d["c"], d["i"]
                    if i == 0:
                        hsums[c] = hsp.tile([128, 2, CH], BF16, tag="hsum",
                                            name="hsum")
                    h2r = h2.rearrange("c k (g d) -> c k g d", d=NDATA)
                    part = hid.tile([128, 2, GPNI, NDATA // 2], BF16,
                                    tag="part", name="part")
                    par2 = hid.tile([128, 2, GPNI, NDATA // 4], BF16,
                                    tag="par2", name="par2")
                    with nc.allow_low_precision(reason="bf16 partial sums"):
                        nc.gpsimd.tensor_tensor(
                            out=part, in0=h2r[:, :, :, 0:NDATA // 2],
                            in1=h2r[:, :, :, NDATA // 2:NDATA], op=ADD)
                        nc.gpsimd.tensor_tensor(
                            out=par2, in0=part[:, :, :, 0:NDATA // 4],
                            in1=part[:, :, :, NDATA // 4:NDATA // 2], op=ADD)
                        nc.vector.reduce_sum(
                            out=hsums[c][:, :, i * GPNI:(i + 1) * GPNI],
                            in_=par2.rearrange("c k g d -> c (k g) d"),
                            axis=AXX)
                else:
                    return h2
                return h2

            h2s = {}

            def emit_l3(t):
                d = seq[t]
                if d["kind"] == "n":
                    if d["last"]:
                        c = d["c"]
                        l3 = ps3.tile([ODE, CH], F32, tag="l3f", name="l3f", bufs=1)
                        for k in (0, 1):
                            nc.tensor.matmul(l3, w["nw3"][:, k],
                                             hsums[c][:, k],
                                             start=(k == 0), stop=(k == 1))
                        nc.scalar.activation(feT[:, c * CH:(c + 1) * CH], l3,
                                             IDENT, bias=w["nb3"])
                    return
                i = d["i"]
                j = i % 2
                h2 = h2s.pop(t)
                if j == 0:
                    l3ps[t] = ps3.tile([128, TN], F32, tag="l3", name="l3")
                l3 = l3ps[t - j]
                for k in (0, 1):
                    nc.tensor.matmul(l3[64 * j:64 * (j + 1)], w["ew3"][:, k],
                                     h2[:, k], start=(k == 0), stop=(k == 1))
                if j == 1:
                    del l3ps[t - 1]
                    e0 = d["c"] * CH * EPG + i * TN
                    ot = oot.tile([128, TN], BF16, tag="ot", name="ot")
                    with nc.allow_low_precision(reason="bf16 out, bias on host"):
                        nc.scalar.activation(
                            ot, l3, mybir.ActivationFunctionType.Copy)
                    nc.sync.dma_start(outT_d[:, e0 - TN:e0], ot[0:64])
                    nc.sync.dma_start(outT_d[:, e0:e0 + TN], ot[64:128])

            for t in range(-2, len(seq) + 1):
                if 0 <= t < len(seq):
                    h2s[t] = emit_l2(t)
                if 0 <= t - 1 < len(seq):
                    emit_l3(t - 1)
                if t + 2 < len(seq):
                    emit_l1(t + 2)
    nc.finalize()
    return nc


# ---------------- general fallback (arbitrary edge_index/batch) ----------------

def _emit_node_stage(nc, pools, w, xT_d, hsum):
    consts, xin, hid, ps1, ps2, ps3 = pools
    GT = TN // NDATA
    for p in range(RC // (2 * TN)):
        r0 = p * 2 * TN
        xtp = xin.tile([128, 2, TN], BF16, tag="xt")
        nc.sync.dma_start(xtp, xT_d[:, r0:r0 + 2 * TN].rearrange("c (t e) -> c t e", t=2))
        h1p = hid.tile([128, 2, 2, TN], BF16, tag="h1")
        for t01 in (0, 1):
            ps_a = ps1.tile([128, TN], F32, tag="l1a")
            ps_b = ps1.tile([128, TN], F32, tag="l1b")
            nc.tensor.matmul(ps_a, w["nw1"][:, 0:128], xtp[:, t01], start=True, stop=True)
            nc.tensor.matmul(ps_b, w["nw1"][:, 128:256], xtp[:, t01], start=True, stop=True)
            nc.scalar.activation(h1p[:, 0, t01], ps_a, RELU, bias=w["nb1"][:, 0:1])
            nc.vector.tensor_scalar(
                out=h1p[:, 1, t01], in0=ps_b, scalar1=w["nb1"][:, 1:2], scalar2=0.0,
                op0=ADD, op1=MAX,
            )
        l2ap = ps2.tile([128, 2, TN], F32, tag="l2a")
        l2bp = ps2.tile([128, 2, TN], F32, tag="l2b")
        for t01 in (0, 1):
            for k in (0, 1):
                nc.tensor.matmul(l2ap[:, t01], w["nw2"][:, k, 0:128], h1p[:, k, t01],
                                 start=(k == 0), stop=(k == 1))
            for k in (0, 1):
                nc.tensor.matmul(l2bp[:, t01], w["nw2"][:, k, 128:256], h1p[:, k, t01],
                                 start=(k == 0), stop=(k == 1))
        h2p = hid.tile([128, 2, 2, TN], BF16, tag="h2")
        nc.scalar.activation(h2p[:, 0], l2ap, RELU, bias=w["nb2"][:, 0:1])
        nc.vector.tensor_scalar(
            out=h2p[:, 1], in0=l2bp, scalar1=w["nb2"][:, 1:2], scalar2=0.0,
            op0=ADD, op1=MAX,
        )
        with nc.allow_low_precision(reason="bf16 reduce feeds bf16 matmul"):
            nc.vector.reduce_sum(
                out=hsum[:, :, p * 2 * GT:(p + 1) * 2 * GT],
                in_=h2p.rearrange("c k t (g d) -> c (k t g) d", d=NDATA),
                axis=AXX,
            )
    ps_f = ps3.tile([ODE, 2, TN], F32, tag="l3")
    for k in (0, 1):
        nc.tensor.matmul(ps_f[:, 0], w["nw3"][:, k], hsum[:, k],
                         start=(k == 0), stop=(k == 1))
    return ps_f[:, 0]


def _emit_edge_stage(nc, pools, w, attrT_d, outT_d, feTg_d):
    consts, xin, hid, ps1, ps2, ps3 = pools
    for p in range(EC // (2 * TN)):
        e0 = p * 2 * TN
        rtp = xin.tile([128, 2, TN], BF16, tag="rt")
        nc.sync.dma_start(rtp[64:128],
                          attrT_d[:, e0:e0 + 2 * TN].rearrange("c (t e) -> c t e", t=2))
        nc.sync.dma_start(rtp[0:64],
                          feTg_d[:, e0:e0 + 2 * TN].rearrange("c (t e) -> c t e", t=2))
        e1p = hid.tile([128, 2, 2, TN], BF16, tag="h1")
        for t01 in (0, 1):
            ps_a = ps1.tile([128, TN], F32, tag="l1a")
            ps_b = ps1.tile([128, TN], F32, tag="l1b")
            nc.tensor.matmul(ps_a, w["ew1"][:, 0:128], rtp[:, t01], start=True, stop=True)
            nc.tensor.matmul(ps_b, w["ew1"][:, 128:256], rtp[:, t01], start=True, stop=True)
            nc.scalar.activation(e1p[:, 0, t01], ps_a, RELU, bias=w["eb1"][:, 0:1])
            nc.vector.tensor_scalar(
                out=e1p[:, 1, t01], in0=ps_b, scalar1=w["eb1"][:, 1:2], scalar2=0.0,
                op0=ADD, op1=MAX,
            )
        l2ap = ps2.tile([128, 2, TN], F32, tag="l2a")
        l2bp = ps2.tile([128, 2, TN], F32, tag="l2b")
        for t01 in (0, 1):
            for k in (0, 1):
                nc.tensor.matmul(l2ap[:, t01], w["ew2"][:, k, 0:128], e1p[:, k, t01],
                                 start=(k == 0), stop=(k == 1))
            for k in (0, 1):
                nc.tensor.matmul(l2bp[:, t01], w["ew2"][:, k, 128:256], e1p[:, k, t01],
                                 start=(k == 0), stop=(k == 1))
        e2p = hid.tile([128, 2, 2, TN], BF16, tag="h2")
        nc.scalar.activation(e2p[:, 0], l2ap, RELU, bias=w["eb2"][:, 0:1])
        nc.vector.tensor_scalar(
            out=e2p[:, 1], in0=l2bp, scalar1=w["eb2"][:, 1:2], scalar2=0.0,
            op0=ADD, op1=MAX,
        )
        l3p = ps3.tile([ODE, 2, TN], F32, tag="l3")
        for t01 in (0, 1):
            for k in (0, 1):
                nc.tensor.matmul(l3p[:, t01], w["ew3"][:, k], e2p[:, k, t01],
                                 start=(k == 0), stop=(k == 1))
        otp = hid.tile([ODE, 2, TN], F32, tag="ot")
        if p % 2 == 0:
            nc.scalar.activation(otp, l3p, IDENT, bias=w["eb3"])
        else:
            nc.vector.tensor_scalar(out=otp, in0=l3p, scalar1=w["eb3"],
                                    scalar2=0.0, op0=ADD, op1=mybir.AluOpType.bypass)
        nc.sync.dma_start(outT_d[:, e0:e0 + 2 * TN],
                          otp.rearrange("c t e -> c (t e)"))


def _build_general(mode):
    """mode: 'node' or 'edge' (general fallback path)."""
    nc = bacc.Bacc("TRN2", target_bir_lowering=False)
    td = _declare_weights(nc)
    if mode == "node":
        xT_d = nc.dram_tensor("xT", [128, RC], BF16, kind="ExternalInput")
        feT_out = nc.dram_tensor("feT", [ODE, GC], F32, kind="ExternalOutput")
    else:
        attrT_d = nc.dram_tensor("attrT", [64, EC], BF16, kind="ExternalInput")
        outT_d = nc.dram_tensor("outT", [64, EC], F32, kind="ExternalOutput")
        feTg_d = nc.dram_tensor("feTg", [64, EC], BF16, kind="ExternalInput")

    with tile.TileContext(nc) as tc:
        with (
            tc.tile_pool(name="consts", bufs=1) as consts,
            tc.tile_pool(name="xin", bufs=4) as xin,
            tc.tile_pool(name="hid", bufs=3) as hid,
            tc.tile_pool(name="ps1", bufs=1, space="PSUM") as ps1,
            tc.tile_pool(name="ps2", bufs=1, space="PSUM") as ps2,
            tc.tile_pool(name="ps3", bufs=2, space="PSUM") as ps3,
        ):
            pools = (consts, xin, hid, ps1, ps2, ps3)
            w = _load_weights(nc, consts, td, node=mode == "node", edge=mode == "edge")
            if mode == "node":
                hsum = consts.tile([128, 2, GC], BF16, tag="hsum")
                ps_f = _emit_node_stage(nc, pools, w, xT_d, hsum)
                feT_sb = consts.tile([ODE, GC], F32, tag="feT")
                nc.scalar.activation(feT_sb, ps_f, IDENT, bias=w["nb3"])
                nc.sync.dma_start(feT_out[:], feT_sb)
            else:
                _emit_edge_stage(nc, pools, w, attrT_d, outT_d, feTg_d)
    nc.finalize()
    return nc


def _get_program(mode):
    if mode not in _PROGRAMS:
        if mode == "fused3":
            _PROGRAMS[mode] = _build_fused3()
        elif mode == "fused2":
            _PROGRAMS[mode] = _build_fused2()
        else:
            _PROGRAMS[mode] = _build_general(mode)
    return _PROGRAMS[mode]


def _shared_weight_arrays(kw):
    f = np.float32
    c = np.ascontiguousarray
    return {
        "nw1": c(np.asarray(kw["node_w1"], dtype=f).astype(BF16NP)),
        "nw2": c(np.asarray(kw["node_w2"], dtype=f).reshape(2, 128, H).transpose(1, 0, 2).astype(BF16NP)),
        "nw3": c(np.asarray(kw["node_w3"], dtype=f).reshape(2, 128, ODE).transpose(1, 0, 2).astype(BF16NP)),
        "nb1": c(np.asarray(kw["node_b1"], dtype=f).reshape(2, 128).T),
        "nb2": c(np.asarray(kw["node_b2"], dtype=f).reshape(2, 128).T),
        "nb3": c(np.asarray(kw["node_b3"], dtype=f).reshape(ODE, 1)),
        "ew1": c(np.asarray(kw["edge_w1"], dtype=f).astype(BF16NP)),
        "ew2": c(np.asarray(kw["edge_w2"], dtype=f).reshape(2, 128, H).transpose(1, 0, 2).astype(BF16NP)),
        "ew3": c(np.asarray(kw["edge_w3"], dtype=f).reshape(2, 128, ODE).transpose(1, 0, 2).astype(BF16NP)),
        "eb1": c(np.asarray(kw["edge_b1"], dtype=f).reshape(2, 128).T),
        "eb2": c(np.asarray(kw["edge_b2"], dtype=f).reshape(2, 128).T),
        "eb3": c(np.asarray(kw["edge_b3"], dtype=f).reshape(EA, 1)),
    }


def _pack_weights(shared):
    wp = np.zeros((128, _W_COLS), dtype=BF16NP)
    wp[:, _W_NW1:_W_NW1 + 256] = shared["nw1"]
    wp[:, _W_NW2:_W_NW2 + 512] = shared["nw2"].reshape(128, 512)
    wp[:, _W_NW3:_W_NW3 + 128] = shared["nw3"].reshape(128, 128)
    wp[:, _W_EW1:_W_EW1 + 256] = shared["ew1"]
    wp[:, _W_EW2:_W_EW2 + 512] = shared["ew2"].reshape(128, 512)
    wp[:, _W_EW3:_W_EW3 + 128] = shared["ew3"].reshape(128, 128)
    return np.ascontiguousarray(wp)


def _x_transposed_per_core(x, c):
    xs = np.asarray(x, dtype=np.float32).reshape(G, ODE, 2, NDATA)[c * GC:(c + 1) * GC]
    return np.ascontiguousarray(xs.transpose(1, 2, 0, 3).reshape(128, RC).astype(BF16NP))


def kernel(x, edge_attr, node_w1, node_b1, node_w2, node_b2, node_w3, node_b3,
           edge_w1, edge_b1, edge_w2, edge_b2, edge_w3, edge_b3,
           edge_index, batch):
    global last_results
    kw = dict(x=x, node_w1=node_w1, node_b1=node_b1, node_w2=node_w2,
              node_b2=node_b2, node_w3=node_w3, node_b3=node_b3,
              edge_w1=edge_w1, edge_b1=edge_b1, edge_w2=edge_w2,
              edge_b2=edge_b2, edge_w3=edge_w3, edge_b3=edge_b3)
    trace = os.environ.get("KERNEL_TRACE", "") == "1"
    if trace:
        _install_trace_shim()

    edge_attr = np.asarray(edge_attr, dtype=np.float32)
    ei = np.asarray(edge_index)
    bt = np.asarray(batch)
    g_src = bt[ei[0]]
    g_dst = bt[ei[1]]
    same = g_src == g_dst
    structured = bool((g_src == np.repeat(np.arange(G), EPG)).all())
    zero_bias = all(
        not np.any(np.asarray(kw[k], dtype=np.float32))
        for k in ("node_b1", "node_b2", "node_b3", "edge_b1", "edge_b2",
                  "edge_b3"))

    shared = _shared_weight_arrays(kw)
    run_kwargs = dict(core_ids=list(range(NCORES)), trace=trace,
                      trace_cores=[0] if trace else None)

    if structured and zero_bias and os.environ.get("KERNEL_FORCE", "") != "fused2":
        nc = _get_program("fused3")
        wp = _pack_weights(shared)
        in_maps = []
        for c in range(NCORES):
            m = {"wpack": wp}
            m["xT"] = _x_transposed_per_core(x, c)
            m["attrT"] = np.ascontiguousarray(edge_attr[c * EC:(c + 1) * EC].T.astype(BF16NP))
            in_maps.append(m)
        res = run_bass_kernel_spmd(nc, in_maps, **run_kwargs)
        last_results = res
        out = np.empty((E, EA), dtype=np.float32)
        for c in range(NCORES):
            out[c * EC:(c + 1) * EC] = res.results[c]["outT"].T.astype(np.float32)
    elif structured:
        nc = _get_program("fused2")
        eb3_host = np.asarray(kw["edge_b3"], dtype=np.float32).reshape(1, EA)
        in_maps = []
        for c in range(NCORES):
            m = {k: v for k, v in shared.items() if k != "eb3"}
            m["xT"] = _x_transposed_per_core(x, c)
            m["attrT"] = np.ascontiguousarray(edge_attr[c * EC:(c + 1) * EC].T.astype(BF16NP))
            in_maps.append(m)
        res = run_bass_kernel_spmd(nc, in_maps, **run_kwargs)
        last_results = res
        out = np.empty((E, EA), dtype=np.float32)
        for c in range(NCORES):
            out[c * EC:(c + 1) * EC] = res.results[c]["outT"].T.astype(np.float32)
        out += eb3_host
    else:
        # general path: node stage -> host gather of feature_enc -> edge stage
        nc_node = _get_program("node")
        in_maps = []
        for c in range(NCORES):
            m = dict(shared)
            m["xT"] = _x_transposed_per_core(x, c)
            in_maps.append(m)
        res_n = run_bass_kernel_spmd(nc_node, in_maps, **run_kwargs)
        feT_full = np.concatenate([res_n.results[c]["feT"] for c in range(NCORES)],
                                  axis=1)          # [64, G]
        feTg = feT_full[:, g_src]                   # [64, E]
        nc_edge = _get_program("edge")
        in_maps = []
        for c in range(NCORES):
            m = dict(shared)
            m["attrT"] = np.ascontiguousarray(edge_attr[c * EC:(c + 1) * EC].T.astype(BF16NP))
            m["feTg"] = np.ascontiguousarray(feTg[:, c * EC:(c + 1) * EC].astype(BF16NP))
            in_maps.append(m)
        res = run_bass_kernel_spmd(nc_edge, in_maps, **run_kwargs)
        last_results = res
        out = np.empty((E, EA), dtype=np.float32)
        for c in range(NCORES):
            out[c * EC:(c + 1) * EC] = res.results[c]["outT"].T
    if not same.all():
        out = np.where(same[:, None], out, edge_attr)
    return out


# revision 55
# speedup vs baseline: 1.0874x; 1.0874x over previous
"""Trainium2 Bass kernel for nn_NodeNet (GNN message passing).

Strategy: data-parallel over graphs across 8 NeuronCores. Host transposes
inputs into [feature, row] layouts so every DMA is contiguous; all matmuls
run in bf16 with transposed activations:
  node stage: dp[128, rows] -> MLP -> sum over datapoints -> feature_enc[64, G]
  edge stage: rhs = [fe (broadcast per graph); edge_attr^T] -> MLP -> out^T

fused3 (fast path, requires structured edges + all-zero biases, which is what
the reference's setup_inputs produces):
  - one merged PSUM tile per MLP layer ([128, 2, TN] spanning 2 banks) so the
    psum->sbuf relu is a single instruction per layer instead of two
  - static engine assignment (found by measurement; alternation/splitting all
    regressed): h1 relu on ACT, h2 relu on DVE, output copy on ACT, fe
    broadcast as ONE DVE copy per pair (a full-size broadcast op costs 674ns;
    any sliced variant costs 1-1.6us, and gpsimd broadcasts contend with DVE
    regardless of source region), node partial sums on gpsimd + DVE reduce
  - all weights packed into one DRAM tensor; the bulk DMA is issued after the
    first x tiles so the first matmul starts ~2us earlier
  - software pipeline emits [L3(t-1), prep(t+4), L2(t), L1(t+2)]; the L3 pair
    is emitted fused at the odd iteration with same-weight matmuls adjacent
  - PSUM: l1 bufs=1 (2 banks) + l2 bufs=2 (4) + l3 bufs=2 (2, shared with the
    node-stage L3) = exactly 8 banks; ps2 single-buffering loses ~200us
fused2 (previous structured path, used when biases are nonzero) and a fully
general two-launch fallback (arbitrary edge_index/batch) are kept below.
"""

import os
import sys

import ml_dtypes
import numpy as np

BF16NP = ml_dtypes.bfloat16

if "/opt/trn_rl_repo" not in sys.path and os.path.isdir("/opt/trn_rl_repo"):
    sys.path.insert(0, "/opt/trn_rl_repo")

import concourse.bacc as bacc
import concourse.tile as tile
from concourse import mybir
from concourse.bass_utils import run_bass_kernel_spmd

G, ODE, NDATA, H, EA, EPG = 4096, 64, 32, 256, 64, 128
E = G * EPG
NCORES = 8
GC = G // NCORES           # graphs per core
RC = GC * NDATA            # node-MLP rows per core
EC = GC * EPG              # edges per core
TN = 512                   # tile free size
CH = 64                    # graphs per interleave chunk
NCH = GC // CH             # chunks per core (8)
NNI = CH * NDATA // TN     # node iters per chunk (4)
NEI = CH * EPG // TN       # edge iters per chunk (16)
GPEI = TN // EPG           # graphs per edge iter (4)
GPNI = TN // NDATA         # graphs per node iter (16)

F32 = mybir.dt.float32
BF16 = mybir.dt.bfloat16
RELU = mybir.ActivationFunctionType.Relu
IDENT = mybir.ActivationFunctionType.Identity
COPY = mybir.ActivationFunctionType.Copy
ADD = mybir.AluOpType.add
MAX = mybir.AluOpType.max
AXX = mybir.AxisListType.X

_PROGRAMS = {}
last_results = None


def _install_trace_shim():
    """Optional: make trace=True work by injecting antenv.axon_hooks."""
    import types

    if "antenv.axon_hooks" in sys.modules:
        return
    try:
        mod = types.ModuleType("antenv.axon_hooks")
        mod._hook = None
        mod.set_axon_ntff_profile_hook = lambda h: setattr(mod, "_hook", h)
        mod.get_axon_ntff_profile_hook = lambda: mod._hook
        sys.modules["antenv.axon_hooks"] = mod
        import antenv

        antenv.axon_hooks = mod
        from trn_agent_boot.trn_boot import _ntff_profile_via_ctypes

        hook = _ntff_profile_via_ctypes("/opt/axon/libaxon_pjrt.so")
        if hook is not None:
            mod.set_axon_ntff_profile_hook(hook)
    except Exception:
        pass


# ------------------------- fused3: zero-bias fast path -------------------------

# packed weight column offsets: nw1 | nw2(k,m) | nw3(k) | ew1 | ew2(k,m) | ew3(k)
_W_NW1 = 0
_W_NW2 = 256
_W_NW3 = 768
_W_EW1 = 896
_W_EW2 = 1152
_W_EW3 = 1664
_W_COLS = 1792


def _build_fused3():
    nc = bacc.Bacc("TRN2", target_bir_lowering=False)
    wp_d = nc.dram_tensor("wpack", [128, _W_COLS], BF16, kind="ExternalInput")
    xT_d = nc.dram_tensor("xT", [128, RC], BF16, kind="ExternalInput")
    attrT_d = nc.dram_tensor("attrT", [64, EC], BF16, kind="ExternalInput")
    outT_d = nc.dram_tensor("outT", [64, EC], BF16, kind="ExternalOutput")

    with tile.TileContext(nc) as tc:
        with (
            tc.tile_pool(name="consts", bufs=1) as consts,
            tc.tile_pool(name="xin", bufs=8) as xin,
            tc.tile_pool(name="hid", bufs=3) as hid,
            tc.tile_pool(name="hsp", bufs=2) as hsp,
            tc.tile_pool(name="oot", bufs=4) as oot,
            tc.tile_pool(name="ps1", bufs=1, space="PSUM") as ps1,
            tc.tile_pool(name="ps2", bufs=2, space="PSUM") as ps2,
            tc.tile_pool(name="ps3", bufs=2, space="PSUM") as ps3,
        ):
            wp = consts.tile([128, _W_COLS], BF16, tag="wp", name="wp")
            nc.sync.dma_start(wp[:, 0:256], wp_d[:, 0:256])
            feT = consts.tile([ODE, GC], BF16, tag="feT")

            def w1(base, m):  # [128, 128] lhsT for layer-1 M-half m
                return wp[:, base + m * 128:base + (m + 1) * 128]

            def w2(base, k, m):
                return wp[:, base + k * 256 + m * 128:base + k * 256 + (m + 1) * 128]

            def w3(base, k):
                return wp[:, base + k * ODE:base + (k + 1) * ODE]

            # ---- interleaved iteration sequence ----
            seq = []

            def n_iter(c, i):
                return dict(kind="n", c=c, i=i, last=(i == NNI - 1))

            def e_iter(c, i):
                return dict(kind="e", c=c, i=i)

            for i in range(NNI):
                seq.append(n_iter(0, i))
            for c in range(NCH):
                nxt = list(range(NNI)) if c + 1 < NCH else []
                k = 0
                lead = 4 if c == 0 else 1
                for _ in range(lead):
                    if k < len(nxt):
                        seq.append(n_iter(c + 1, nxt[k]))
                        k += 1
                for i in range(NEI):
                    seq.append(e_iter(c, i))
                    if i % 4 == 3 and k < len(nxt):
                        seq.append(n_iter(c + 1, nxt[k]))
                        k += 1
                while k < len(nxt):
                    seq.append(n_iter(c + 1, nxt[k]))
                    k += 1

            # ---- ACT/DVE greedy load balancer ----
            bal = {"A": 0.0, "V": 0.0}

            def pick_engine(cost_a, cost_v):
                if bal["A"] + cost_a <= bal["V"] + cost_v:
                    bal["A"] += cost_a
                    return "A"
                bal["V"] += cost_v
                return "V"

            def emit_relu(out, in_, nels, force=None):
                ca = 220 + 0.85 * nels
                cv = 220 + 1.06 * nels
                if force == "A":
                    bal["A"] += ca
                    e = "A"
                elif force == "V":
                    bal["V"] += cv
                    e = "V"
                else:
                    e = pick_engine(ca, cv)
                if e == "A":
                    nc.scalar.activation(out, in_, RELU)
                else:
                    nc.vector.tensor_scalar(
                        out=out, in0=in_, scalar1=0.0, scalar2=None, op0=MAX)
                return "V" if e == "A" else "A"

            def emit_copy(out, in_, nels):
                bal["A"] += 220 + 0.85 * nels
                with nc.allow_low_precision(reason="bf16 out, zero bias"):
                    nc.scalar.activation(out, in_, COPY)

            rts = {}      # t -> input tile (node) / paired input tile (edge even i)
            h1s = {}      # t -> h1 sbuf tile
            h2s = {}      # t -> h2 sbuf tile
            l3ps = {}     # t (even local i) -> shared l3 psum tile
            hsums = {}
            nprep = [0]
            deferred = []

            def prep(t, step):
                d = seq[t]
                if d["kind"] == "n":
                    col0 = d["c"] * CH * NDATA + d["i"] * TN
                    xt = xin.tile([128, TN], BF16, tag="xt", name="xt")
                    nc.sync.dma_start(xt, xT_d[:, col0:col0 + TN])
                    rts[t] = xt
                elif d["i"] % 2 == 0:
                    e0 = d["c"] * CH * EPG + d["i"] * TN
                    g0 = d["c"] * CH + d["i"] * GPEI
                    rt = xin.tile([128, 2, TN], BF16, tag="rt", name="rt")
                    nc.sync.dma_start(
                        rt[64:128], attrT_d[:, e0:e0 + 2 * TN].rearrange(
                            "c (t e) -> c t e", t=2))
                    dst = rt[0:64].rearrange("c t (g e) -> c (t g) e", e=EPG)
                    src = feT[:, g0:g0 + 2 * GPEI, None].to_broadcast(
                        [ODE, 2 * GPEI, EPG])
                    nc.vector.tensor_copy(out=dst, in_=src)
                    nprep[0] += 1
                    rts[t] = rt

            def emit_l1(t, step):
                d = seq[t]
                n = d["kind"] == "n"
                base = _W_NW1 if n else _W_EW1
                if n:
                    mv = rts.pop(t)
                else:
                    j = d["i"] % 2
                    mv = rts[t - j][:, j]
                    if j == 1:
                        del rts[t - 1]
                l1 = ps1.tile([128, 2, TN], F32, tag="l1", name="l1")
                nc.tensor.matmul(l1[:, 0], w1(base, 0), mv, start=True, stop=True)
                nc.tensor.matmul(l1[:, 1], w1(base, 1), mv, start=True, stop=True)
                h1 = hid.tile([128, 2, TN], BF16, tag="h1", name="h1")
                d["h1eng"] = emit_relu(h1, l1, 1024, force="A")
                h1s[t] = h1

            def emit_l2(t, step):
                d = seq[t]
                n = d["kind"] == "n"
                base = _W_NW2 if n else _W_EW2
                h1 = h1s.pop(t)
                l2 = ps2.tile([128, 2, TN], F32, tag="l2", name="l2")
                for m in (0, 1):
                    for k in (0, 1):
                        nc.tensor.matmul(
                            l2[:, m], w2(base, k, m), h1[:, k],
                            start=(k == 0), stop=(k == 1))
                h2 = hid.tile([128, 2, TN], BF16, tag="h2", name="h2")
                emit_relu(h2, l2, 1024, force=d.get("h1eng"))
                if n:
                    c, i = d["c"], d["i"]
                    if i == 0:
                        hsums[c] = hsp.tile([128, 2, CH], BF16, tag="hsum",
                                            name="hsum")
                    h2r = h2.rearrange("c k (g d) -> c k g d", d=NDATA)
                    part = hid.tile([128, 2, GPNI, NDATA // 2], BF16,
                                    tag="part", name="part")
                    par2 = hid.tile([128, 2, GPNI, NDATA // 4], BF16,
                                    tag="par2", name="par2")
                    with nc.allow_low_precision(reason="bf16 partial sums"):
                        nc.gpsimd.tensor_tensor(
                            out=part, in0=h2r[:, :, :, 0:NDATA // 2],
                            in1=h2r[:, :, :, NDATA // 2:NDATA], op=ADD)
                        nc.gpsimd.tensor_tensor(
                            out=par2, in0=part[:, :, :, 0:NDATA // 4],
                            in1=part[:, :, :, NDATA // 4:NDATA // 2], op=ADD)
                        nc.vector.reduce_sum(
                            out=hsums[c][:, :, i * GPNI:(i + 1) * GPNI],
                            in_=par2.rearrange("c k g d -> c (k g) d"),
                            axis=AXX)
                h2s[t] = h2

            def emit_l3(t, step):
                d = seq[t]
                if d["kind"] == "n":
                    h2s.pop(t, None)
                    if d["last"]:
                        c = d["c"]
                        l3n = ps3.tile([128, TN], F32, tag="l3", name="l3")
                        for k in (0, 1):
                            nc.tensor.matmul(l3n[0:ODE, 0:CH], w3(_W_NW3, k),
                                             hsums[c][:, k],
                                             start=(k == 0), stop=(k == 1))
                        with nc.allow_low_precision(reason="bf16 feT"):
                            nc.scalar.activation(
                                feT[:, c * CH:(c + 1) * CH], l3n[0:ODE, 0:CH],
                                COPY)
                    return
                i = d["i"]
                j = i % 2
                if j == 0:
                    return
                h2a = h2s.pop(t - 1)
                h2b = h2s.pop(t)
                l3 = ps3.tile([128, TN], F32, tag="l3", name="l3")
                # same-weight matmuls adjacent: k0 over both pair halves, then k1
                for k in (0, 1):
                    nc.tensor.matmul(l3[0:64], w3(_W_EW3, k), h2a[:, k],
                                     start=(k == 0), stop=(k == 1))
                    nc.tensor.matmul(l3[64:128], w3(_W_EW3, k), h2b[:, k],
                                     start=(k == 0), stop=(k == 1))
                e0 = d["c"] * CH * EPG + i * TN
                ot = oot.tile([128, TN], BF16, tag="ot", name="ot")
                emit_copy(ot, l3, 512)
                nc.sync.dma_start(outT_d[:, e0 - TN:e0], ot[0:64])
                nc.sync.dma_start(outT_d[:, e0:e0 + TN], ot[64:128])

            NT = len(seq)
            for t in range(-4, NT + 1):
                if 0 <= t - 1 < NT:
                    emit_l3(t - 1, t)
                if 0 <= t + 4 < NT:
                    prep(t + 4, t)
                if 0 <= t < NT:
                    emit_l2(t, t)
                if 0 <= t + 2 < NT:
                    emit_l1(t + 2, t)
                if t == -2:
                    nc.sync.dma_start(wp[:, 256:_W_COLS], wp_d[:, 256:_W_COLS])
    nc.finalize()
    return nc


# ------------------------- fused2: structured path with biases ----------------

def _declare_weights(nc, with_eb3=True):
    t = {}
    t["nw1"] = nc.dram_tensor("nw1", [128, H], BF16, kind="ExternalInput")
    t["nw2"] = nc.dram_tensor("nw2", [128, 2, H], BF16, kind="ExternalInput")
    t["nw3"] = nc.dram_tensor("nw3", [128, 2, ODE], BF16, kind="ExternalInput")
    t["nb1"] = nc.dram_tensor("nb1", [128, 2], F32, kind="ExternalInput")
    t["nb2"] = nc.dram_tensor("nb2", [128, 2], F32, kind="ExternalInput")
    t["nb3"] = nc.dram_tensor("nb3", [ODE, 1], F32, kind="ExternalInput")
    t["ew1"] = nc.dram_tensor("ew1", [128, H], BF16, kind="ExternalInput")
    t["ew2"] = nc.dram_tensor("ew2", [128, 2, H], BF16, kind="ExternalInput")
    t["ew3"] = nc.dram_tensor("ew3", [128, 2, ODE], BF16, kind="ExternalInput")
    t["eb1"] = nc.dram_tensor("eb1", [128, 2], F32, kind="ExternalInput")
    t["eb2"] = nc.dram_tensor("eb2", [128, 2], F32, kind="ExternalInput")
    if with_eb3:
        t["eb3"] = nc.dram_tensor("eb3", [EA, 1], F32, kind="ExternalInput")
    return t


def _load_weights(nc, consts, td, node: bool, edge: bool, with_eb3=True):
    sb = {}
    names = []
    if node:
        names += ["nw1", "nw2", "nw3", "nb1", "nb2", "nb3"]
    if edge:
        names += ["ew1", "ew2", "ew3", "eb1", "eb2"]
        if with_eb3:
            names += ["eb3"]
    for n in names:
        d = td[n]
        sb[n] = consts.tile(list(d.shape), d.dtype, tag=n, name=n)
        nc.sync.dma_start(sb[n], d[:])
    return sb


def _build_fused2():
    """Structured path with bias support (see git history for details)."""
    nc = bacc.Bacc("TRN2", target_bir_lowering=False)
    td = _declare_weights(nc, with_eb3=False)
    xT_d = nc.dram_tensor("xT", [128, RC], BF16, kind="ExternalInput")
    attrT_d = nc.dram_tensor("attrT", [64, EC], BF16, kind="ExternalInput")
    outT_d = nc.dram_tensor("outT", [64, EC], BF16, kind="ExternalOutput")

    with tile.TileContext(nc) as tc:
        with (
            tc.tile_pool(name="consts", bufs=1) as consts,
            tc.tile_pool(name="xin", bufs=3) as xin,
            tc.tile_pool(name="hid", bufs=3) as hid,
            tc.tile_pool(name="oot", bufs=3) as oot,
            tc.tile_pool(name="hsp", bufs=2) as hsp,
            tc.tile_pool(name="ps1", bufs=2, space="PSUM") as ps1,
            tc.tile_pool(name="ps2", bufs=2, space="PSUM") as ps2,
            tc.tile_pool(name="ps3", bufs=2, space="PSUM") as ps3,
        ):
            w = _load_weights(nc, consts, td, node=True, edge=True, with_eb3=False)
            feT = consts.tile([ODE, GC], BF16, tag="feT")

            seq = []
            hsums = {}

            def add_node(c):
                for i in range(NNI):
                    seq.append(dict(kind="n", c=c, i=i,
                                    last=(i == NNI - 1)))

            add_node(0)
            for c in range(NCH):
                for q in range(4):
                    if c + 1 < NCH and not (c == 0 and q == 1):
                        seq.append(dict(kind="n", c=c + 1, i=q,
                                        last=(q == NNI - 1)))
                    if c == 0 and q == 0 and 1 < NCH:
                        seq.append(dict(kind="n", c=1, i=1, last=False))
                    for i in range(4 * q, 4 * q + 4):
                        seq.append(dict(kind="e", c=c, i=i))
                    if c == 0 and q == 0:
                        continue

            rts = {}
            h1s = {}
            l3ps = {}

            def prep(t):
                d = seq[t]
                if d["kind"] == "n":
                    col0 = d["c"] * CH * NDATA + d["i"] * TN
                    xt = xin.tile([128, TN], BF16, tag="xt", name="xt")
                    nc.sync.dma_start(xt, xT_d[:, col0:col0 + TN])
                    rts[t] = xt
                else:
                    e0 = d["c"] * CH * EPG + d["i"] * TN
                    g0 = d["c"] * CH + d["i"] * GPEI
                    rt = xin.tile([128, 2, TN], BF16, tag="rt", name="rt")
                    nc.sync.dma_start(
                        rt[64:128], attrT_d[:, e0:e0 + 2 * TN].rearrange(
                            "c (t e) -> c t e", t=2))
                    nc.vector.tensor_copy(
                        out=rt[0:64].rearrange("c t (g e) -> c (t g) e", e=EPG),
                        in_=feT[:, g0:g0 + 2 * GPEI, None].to_broadcast(
                            [ODE, 2 * GPEI, EPG]))
                    rts[t] = rt

            def emit_l1(t):
                d = seq[t]
                n = d["kind"] == "n"
                w1, b1 = (w["nw1"], w["nb1"]) if n else (w["ew1"], w["eb1"])
                if n:
                    prep(t)
                    mv = rts.pop(t)
                else:
                    if d["i"] % 2 == 0:
                        prep(t)
                    mv = rts[t - d["i"] % 2][:, d["i"] % 2]
                l1a = ps1.tile([128, TN], F32, tag="l1a", name="l1a", bufs=2)
                l1b = ps1.tile([128, TN], F32, tag="l1b", name="l1b", bufs=1)
                nc.tensor.matmul(l1a, w1[:, 0:128], mv, start=True, stop=True)
                nc.tensor.matmul(l1b, w1[:, 128:256], mv, start=True, stop=True)
                h1 = hid.tile([128, 2, TN], BF16, tag="h1", name="h1")
                nc.scalar.activation(h1[:, 0], l1a, RELU, bias=b1[:, 0:1])
                nc.vector.tensor_scalar(
                    out=h1[:, 1], in0=l1b, scalar1=b1[:, 1:2],
                    scalar2=0.0, op0=ADD, op1=MAX)
                h1s[t] = h1

            def emit_l2(t):
                d = seq[t]
                n = d["kind"] == "n"
                w2, b2 = (w["nw2"], w["nb2"]) if n else (w["ew2"], w["eb2"])
                h1 = h1s.pop(t)
                l2a = ps2.tile([128, TN], F32, tag="l2a", name="l2a", bufs=1)
                l2b = ps2.tile([128, TN], F32, tag="l2b", name="l2b", bufs=1)
                for m, lt in ((0, l2a), (1, l2b)):
                    for k in (0, 1):
                        nc.tensor.matmul(
                            lt, w2[:, k, m * 128:(m + 1) * 128],
                            h1[:, k], start=(k == 0), stop=(k == 1))
                h2 = hid.tile([128, 2, TN], BF16, tag="h2", name="h2")
                nc.scalar.activation(h2[:, 0], l2a, RELU, bias=b2[:, 0:1])
                if n and d["i"] % 2 == 0:
                    nc.scalar.activation(h2[:, 1], l2b, RELU,
                                         bias=b2[:, 1:2])
                else:
                    nc.vector.tensor_scalar(
                        out=h2[:, 1], in0=l2b, scalar1=b2[:, 1:2],
                        scalar2=0.0, op0=ADD, op1=MAX)
                if n:
                    c, i = d["c"], d["i"]
                    if i == 0:
                        hsums[c] = hsp.tile([128, 2, CH], BF16, tag="hsum",
                                            name="hsum")
                    h2r = h2.rearrange("c k (g d) -> c k g d", d=NDATA)
                    part = hid.tile([128, 2, GPNI, NDATA // 2], BF16,
                                    tag="part", name="part")
                    par2 = hid.tile([128, 2, GPNI, NDATA // 4], BF16,
                                    tag="par2", name="par2")
                    with nc.allow_low_precision(reason="bf16 partial sums"):
                        nc.gpsimd.tensor_tensor(
                            out=part, in0=h2r[:, :, :, 0:NDATA // 2],
                            in1=h2r[:, :, :, NDATA // 2:NDATA], op=ADD)
                        nc.gpsimd.tensor_tensor(
                            out=par2, in0=part[:, :, :, 0:NDATA // 4],
                            in1=part[:, :, :, NDATA // 4:NDATA // 2], op=ADD)
                        nc.vector.reduce_sum(
                            out=hsums[c][:, :, i * GPNI:(i + 1) * GPNI],
                            in_=par2.rearrange("c k g d -> c (k g) d"),
                            axis=AXX)
                else:
                    return h2
                return h2

            h2s = {}

            def emit_l3(t):
                d = seq[t]
                if d["kind"] == "n":
                    if d["last"]:
                        c = d["c"]
                        l3 = ps3.tile([ODE, CH], F32, tag="l3f", name="l3f", bufs=1)
                        for k in (0, 1):
                            nc.tensor.matmul(l3, w["nw3"][:, k],
                                             hsums[c][:, k],
                                             start=(k == 0), stop=(k == 1))
                        nc.scalar.activation(feT[:, c * CH:(c + 1) * CH], l3,
                                             IDENT, bias=w["nb3"])
                    return
                i = d["i"]
                j = i % 2
                h2 = h2s.pop(t)
                if j == 0:
                    l3ps[t] = ps3.tile([128, TN], F32, tag="l3", name="l3")
                l3 = l3ps[t - j]
                for k in (0, 1):
                    nc.tensor.matmul(l3[64 * j:64 * (j + 1)], w["ew3"][:, k],
                                     h2[:, k], start=(k == 0), stop=(k == 1))
                if j == 1:
                    del l3ps[t - 1]
                    e0 = d["c"] * CH * EPG + i * TN
                    ot = oot.tile([128, TN], BF16, tag="ot", name="ot")
                    with nc.allow_low_precision(reason="bf16 out, bias on host"):
                        nc.scalar.activation(
                            ot, l3, mybir.ActivationFunctionType.Copy)
                    nc.sync.dma_start(outT_d[:, e0 - TN:e0], ot[0:64])
                    nc.sync.dma_start(outT_d[:, e0:e0 + TN], ot[64:128])

            for t in range(-2, len(seq) + 1):
                if 0 <= t < len(seq):
                    h2s[t] = emit_l2(t)
                if 0 <= t - 1 < len(seq):
                    emit_l3(t - 1)
                if t + 2 < len(seq):
                    emit_l1(t + 2)
    nc.finalize()
    return nc


# ---------------- general fallback (arbitrary edge_index/batch) ----------------

def _emit_node_stage(nc, pools, w, xT_d, hsum):
    consts, xin, hid, ps1, ps2, ps3 = pools
    GT = TN // NDATA
    for p in range(RC // (2 * TN)):
        r0 = p * 2 * TN
        xtp = xin.tile([128, 2, TN], BF16, tag="xt")
        nc.sync.dma_start(xtp, xT_d[:, r0:r0 + 2 * TN].rearrange("c (t e) -> c t e", t=2))
        h1p = hid.tile([128, 2, 2, TN], BF16, tag="h1")
        for t01 in (0, 1):
            ps_a = ps1.tile([128, TN], F32, tag="l1a")
            ps_b = ps1.tile([128, TN], F32, tag="l1b")
            nc.tensor.matmul(ps_a, w["nw1"][:, 0:128], xtp[:, t01], start=True, stop=True)
            nc.tensor.matmul(ps_b, w["nw1"][:, 128:256], xtp[:, t01], start=True, stop=True)
            nc.scalar.activation(h1p[:, 0, t01], ps_a, RELU, bias=w["nb1"][:, 0:1])
            nc.vector.tensor_scalar(
                out=h1p[:, 1, t01], in0=ps_b, scalar1=w["nb1"][:, 1:2], scalar2=0.0,
                op0=ADD, op1=MAX,
            )
        l2ap = ps2.tile([128, 2, TN], F32, tag="l2a")
        l2bp = ps2.tile([128, 2, TN], F32, tag="l2b")
        for t01 in (0, 1):
            for k in (0, 1):
                nc.tensor.matmul(l2ap[:, t01], w["nw2"][:, k, 0:128], h1p[:, k, t01],
                                 start=(k == 0), stop=(k == 1))
            for k in (0, 1):
                nc.tensor.matmul(l2bp[:, t01], w["nw2"][:, k, 128:256], h1p[:, k, t01],
                                 start=(k == 0), stop=(k == 1))
        h2p = hid.tile([128, 2, 2, TN], BF16, tag="h2")
        nc.scalar.activation(h2p[:, 0], l2ap, RELU, bias=w["nb2"][:, 0:1])
        nc.vector.tensor_scalar(
            out=h2p[:, 1], in0=l2bp, scalar1=w["nb2"][:, 1:2], scalar2=0.0,
            op0=ADD, op1=MAX,
        )
        with nc.allow_low_precision(reason="bf16 reduce feeds bf16 matmul"):
            nc.vector.reduce_sum(
                out=hsum[:, :, p * 2 * GT:(p + 1) * 2 * GT],
                in_=h2p.rearrange("c k t (g d) -> c (k t g) d", d=NDATA),
                axis=AXX,
            )
    ps_f = ps3.tile([ODE, 2, TN], F32, tag="l3")
    for k in (0, 1):
        nc.tensor.matmul(ps_f[:, 0], w["nw3"][:, k], hsum[:, k],
                         start=(k == 0), stop=(k == 1))
    return ps_f[:, 0]


def _emit_edge_stage(nc, pools, w, attrT_d, outT_d, feTg_d):
    consts, xin, hid, ps1, ps2, ps3 = pools
    for p in range(EC // (2 * TN)):
        e0 = p * 2 * TN
        rtp = xin.tile([128, 2, TN], BF16, tag="rt")
        nc.sync.dma_start(rtp[64:128],
                          attrT_d[:, e0:e0 + 2 * TN].rearrange("c (t e) -> c t e", t=2))
        nc.sync.dma_start(rtp[0:64],
                          feTg_d[:, e0:e0 + 2 * TN].rearrange("c (t e) -> c t e", t=2))
        e1p = hid.tile([128, 2, 2, TN], BF16, tag="h1")
        for t01 in (0, 1):
            ps_a = ps1.tile([128, TN], F32, tag="l1a")
            ps_b = ps1.tile([128, TN], F32, tag="l1b")
            nc.tensor.matmul(ps_a, w["ew1"][:, 0:128], rtp[:, t01], start=True, stop=True)
            nc.tensor.matmul(ps_b, w["ew1"][:, 128:256], rtp[:, t01], start=True, stop=True)
            nc.scalar.activation(e1p[:, 0, t01], ps_a, RELU, bias=w["eb1"][:, 0:1])
            nc.vector.tensor_scalar(
                out=e1p[:, 1, t01], in0=ps_b, scalar1=w["eb1"][:, 1:2], scalar2=0.0,
                op0=ADD, op1=MAX,
            )
        l2ap = ps2.tile([128, 2, TN], F32, tag="l2a")
        l2bp = ps2.tile([128, 2, TN], F32, tag="l2b")
        for t01 in (0, 1):
            for k in (0, 1):
                nc.tensor.matmul(l2ap[:, t01], w["ew2"][:, k, 0:128], e1p[:, k, t01],
                                 start=(k == 0), stop=(k == 1))
            for k in (0, 1):
                nc.tensor.matmul(l2bp[:, t01], w["ew2"][:, k, 128:256], e1p[:, k, t01],
                                 start=(k == 0), stop=(k == 1))
        e2p = hid.tile([128, 2, 2, TN], BF16, tag="h2")
        nc.scalar.activation(e2p[:, 0], l2ap, RELU, bias=w["eb2"][:, 0:1])
        nc.vector.tensor_scalar(
            out=e2p[:, 1], in0=l2bp, scalar1=w["eb2"][:, 1:2], scalar2=0.0,
            op0=ADD, op1=MAX,
        )
        l3p = ps3.tile([ODE, 2, TN], F32, tag="l3")
        for t01 in (0, 1):
            for k in (0, 1):
                nc.tensor.matmul(l3p[:, t01], w["ew3"][:, k], e2p[:, k, t01],
                                 start=(k == 0), stop=(k == 1))
        otp = hid.tile([ODE, 2, TN], F32, tag="ot")
        if p % 2 == 0:
            nc.scalar.activation(otp, l3p, IDENT, bias=w["eb3"])
        else:
            nc.vector.tensor_scalar(out=otp, in0=l3p, scalar1=w["eb3"],
                                    scalar2=0.0, op0=ADD, op1=mybir.AluOpType.bypass)
        nc.sync.dma_start(outT_d[:, e0:e0 + 2 * TN],
                          otp.rearrange("c t e -> c (t e)"))


def _build_general(mode):
    """mode: 'node' or 'edge' (general fallback path)."""
    nc = bacc.Bacc("TRN2", target_bir_lowering=False)
    td = _declare_weights(nc)
    if mode == "node":
        xT_d = nc.dram_tensor("xT", [128, RC], BF16, kind="ExternalInput")
        feT_out = nc.dram_tensor("feT", [ODE, GC], F32, kind="ExternalOutput")
    else:
        attrT_d = nc.dram_tensor("attrT", [64, EC], BF16, kind="ExternalInput")
        outT_d = nc.dram_tensor("outT", [64, EC], F32, kind="ExternalOutput")
        feTg_d = nc.dram_tensor("feTg", [64, EC], BF16, kind="ExternalInput")

    with tile.TileContext(nc) as tc:
        with (
            tc.tile_pool(name="consts", bufs=1) as consts,
            tc.tile_pool(name="xin", bufs=4) as xin,
            tc.tile_pool(name="hid", bufs=3) as hid,
            tc.tile_pool(name="ps1", bufs=1, space="PSUM") as ps1,
            tc.tile_pool(name="ps2", bufs=1, space="PSUM") as ps2,
            tc.tile_pool(name="ps3", bufs=2, space="PSUM") as ps3,
        ):
            pools = (consts, xin, hid, ps1, ps2, ps3)
            w = _load_weights(nc, consts, td, node=mode == "node", edge=mode == "edge")
            if mode == "node":
                hsum = consts.tile([128, 2, GC], BF16, tag="hsum")
                ps_f = _emit_node_stage(nc, pools, w, xT_d, hsum)
                feT_sb = consts.tile([ODE, GC], F32, tag="feT")
                nc.scalar.activation(feT_sb, ps_f, IDENT, bias=w["nb3"])
                nc.sync.dma_start(feT_out[:], feT_sb)
            else:
                _emit_edge_stage(nc, pools, w, attrT_d, outT_d, feTg_d)
    nc.finalize()
    return nc


def _get_program(mode):
    if mode not in _PROGRAMS:
        if mode == "fused3":
            _PROGRAMS[mode] = _build_fused3()
        elif mode == "fused2":
            _PROGRAMS[mode] = _build_fused2()
        else:
            _PROGRAMS[mode] = _build_general(mode)
    return _PROGRAMS[mode]


def _shared_weight_arrays(kw):
    f = np.float32
    c = np.ascontiguousarray
    return {
        "nw1": c(np.asarray(kw["node_w1"], dtype=f).astype(BF16NP)),
        "nw2": c(np.asarray(kw["node_w2"], dtype=f).reshape(2, 128, H).transpose(1, 0, 2).astype(BF16NP)),
        "nw3": c(np.asarray(kw["node_w3"], dtype=f).reshape(2, 128, ODE).transpose(1, 0, 2).astype(BF16NP)),
        "nb1": c(np.asarray(kw["node_b1"], dtype=f).reshape(2, 128).T),
        "nb2": c(np.asarray(kw["node_b2"], dtype=f).reshape(2, 128).T),
        "nb3": c(np.asarray(kw["node_b3"], dtype=f).reshape(ODE, 1)),
        "ew1": c(np.asarray(kw["edge_w1"], dtype=f).astype(BF16NP)),
        "ew2": c(np.asarray(kw["edge_w2"], dtype=f).reshape(2, 128, H).transpose(1, 0, 2).astype(BF16NP)),
        "ew3": c(np.asarray(kw["edge_w3"], dtype=f).reshape(2, 128, ODE).transpose(1, 0, 2).astype(BF16NP)),
        "eb1": c(np.asarray(kw["edge_b1"], dtype=f).reshape(2, 128).T),
        "eb2": c(np.asarray(kw["edge_b2"], dtype=f).reshape(2, 128).T),
        "eb3": c(np.asarray(kw["edge_b3"], dtype=f).reshape(EA, 1)),
    }


def _pack_weights(shared):
    wp = np.zeros((128, _W_COLS), dtype=BF16NP)
    wp[:, _W_NW1:_W_NW1 + 256] = shared["nw1"]
    wp[:, _W_NW2:_W_NW2 + 512] = shared["nw2"].reshape(128, 512)
    wp[:, _W_NW3:_W_NW3 + 128] = shared["nw3"].reshape(128, 128)
    wp[:, _W_EW1:_W_EW1 + 256] = shared["ew1"]
    wp[:, _W_EW2:_W_EW2 + 512] = shared["ew2"].reshape(128, 512)
    wp[:, _W_EW3:_W_EW3 + 128] = shared["ew3"].reshape(128, 128)
    return np.ascontiguousarray(wp)


def _x_transposed_per_core(x, c):
    xs = np.asarray(x, dtype=np.float32).reshape(G, ODE, 2, NDATA)[c * GC:(c + 1) * GC]
    return np.ascontiguousarray(xs.transpose(1, 2, 0, 3).reshape(128, RC).astype(BF16NP))


def kernel(x, edge_attr, node_w1, node_b1, node_w2, node_b2, node_w3, node_b3,
           edge_w1, edge_b1, edge_w2, edge_b2, edge_w3, edge_b3,
           edge_index, batch):
    global last_results
    kw = dict(x=x, node_w1=node_w1, node_b1=node_b1, node_w2=node_w2,
              node_b2=node_b2, node_w3=node_w3, node_b3=node_b3,
              edge_w1=edge_w1, edge_b1=edge_b1, edge_w2=edge_w2,
              edge_b2=edge_b2, edge_w3=edge_w3, edge_b3=edge_b3)
    trace = os.environ.get("KERNEL_TRACE", "") == "1"
    if trace:
        _install_trace_shim()

    edge_attr = np.asarray(edge_attr, dtype=np.float32)
    ei = np.asarray(edge_index)
    bt = np.asarray(batch)
    g_src = bt[ei[0]]
    g_dst = bt[ei[1]]
    same = g_src == g_dst
    structured = bool((g_src == np.repeat(np.arange(G), EPG)).all())
    zero_bias = all(
        not np.any(np.asarray(kw[k], dtype=np.float32))
        for k in ("node_b1", "node_b2", "node_b3", "edge_b1", "edge_b2",
                  "edge_b3"))

    shared = _shared_weight_arrays(kw)
    run_kwargs = dict(core_ids=list(range(NCORES)), trace=trace,
                      trace_cores=[0] if trace else None)

    if structured and zero_bias and os.environ.get("KERNEL_FORCE", "") != "fused2":
        nc = _get_program("fused3")
        wp = _pack_weights(shared)
        in_maps = []
        for c in range(NCORES):
            m = {"wpack": wp}
            m["xT"] = _x_transposed_per_core(x, c)
            m["attrT"] = np.ascontiguousarray(edge_attr[c * EC:(c + 1) * EC].T.astype(BF16NP))
            in_maps.append(m)
        res = run_bass_kernel_spmd(nc, in_maps, **run_kwargs)
        last_results = res
        out = np.empty((E, EA), dtype=np.float32)
        for c in range(NCORES):
            out[c * EC:(c + 1) * EC] = res.results[c]["outT"].T.astype(np.float32)
    elif structured:
        nc = _get_program("fused2")
        eb3_host = np.asarray(kw["edge_b3"], dtype=np.float32).reshape(1, EA)
        in_maps = []
        for c in range(NCORES):
            m = {k: v for k, v in shared.items() if k != "eb3"}
            m["xT"] = _x_transposed_per_core(x, c)
            m["attrT"] = np.ascontiguousarray(edge_attr[c * EC:(c + 1) * EC].T.astype(BF16NP))
            in_maps.append(m)
        res = run_bass_kernel_spmd(nc, in_maps, **run_kwargs)
        last_results = res
        out = np.empty((E, EA), dtype=np.float32)
        for c in range(NCORES):
            out[c * EC:(c + 1) * EC] = res.results[c]["outT"].T.astype(np.float32)
        out += eb3_host
    else:
        # general path: node stage -> host gather of feature_enc -> edge stage
        nc_node = _get_program("node")
        in_maps = []
        for c in range(NCORES):
            m = dict(shared)
            m["xT"] = _x_transposed_per_core(x, c)
            in_maps.append(m)
        res_n = run_bass_kernel_spmd(nc_node, in_maps, **run_kwargs)
        feT_full = np.concatenate([res_n.results[c]["feT"] for c in range(NCORES)],
                                  axis=1)          # [64, G]
        feTg = feT_full[:, g_src]                   # [64, E]
        nc_edge = _get_program("edge")
        in_maps = []
        for c in range(NCORES):
            m = dict(shared)
            m["attrT"] = np.ascontiguousarray(edge_attr[c * EC:(c + 1) * EC].T.astype(BF16NP))
            m["feTg"] = np.ascontiguousarray(feTg[:, c * EC:(c + 1) * EC].astype(BF16NP))
            in_maps.append(m)
        res = run_bass_kernel_spmd(nc_edge, in_maps, **run_kwargs)
        last_results = res
        out = np.empty((E, EA), dtype=np.float32)
        for c in range(NCORES):
            out[c * EC:(c + 1) * EC] = res.results[c]["outT"].T
    if not same.all():
        out = np.where(same[:, None], out, edge_attr)
    return out
